# revision 1
# baseline (speedup 1.0000x reference)
"""Autoregressive GRU on 8 TRN2 NeuronCores.

Problem: B=256, D=1024, T=128 decode steps.
  step:  z = sig(inp@Wz + h@Uz + bz); r = sig(inp@Wr + h@Ur + br)
         hh = tanh(inp@Wh + bh + r*(h@Uh));  h' = z*h + (1-z)*hh
  inp(0) = 0, h(0) = x, and inp(t) == h(t) for t >= 1, so steps >= 2 use the
  fused weights Gz = Wz+Uz, Gr = Wr+Ur (the z/r gates see inp+h through one
  matmul) plus Wh and Uh separately (r gates only the Uh product).

Sharding: 8-way feature parallel, transposed recurrence. Core c owns h-features
[c*128, (c+1)*128). Each step it computes, for its features, the four gate
pre-activations as out[feat(128), batch(256)] = G_tile.T @ hT (weights
stationary on the PE, fp16 in / fp32 psum accumulate), applies the gate math in
fp32, then pushes its updated fp16 hT chunk into the 7 peer cores' SBUF with
single-destination remote_dma sends (64 KB each, SBUF->SBUF, per-pair
remote-semaphore signaled, compile-time slot addresses); its own k-tile is
read straight from the local fp16 state, so the PE starts each step before
any transfer lands. No collectives, no HBM bounce inside the loop.

The 128 steps are fully unrolled; cross-engine/cross-core ordering is explicit
via semaphores (see comments in _build for the protocol invariants).
"""

import numpy as np

B = 256          # batch
D = 1024         # hidden
T = 128          # decode steps
NCORES = 8
FB = D // NCORES  # features per core = 128
KT = D // 128     # k-tiles = 8


def _build(t_steps: int, with_bias: bool, warm_dummies: int = 2):
    import concourse.bass as bass
    import concourse.mybir as mybir
    from concourse import bacc

    f16 = mybir.dt.float16
    f32 = mybir.dt.float32
    Alu = mybir.AluOpType
    Act = mybir.ActivationFunctionType

    nc = bacc.Bacc()

    # ---- external I/O (per core) ----
    # wg:  stationary weight tiles, fp16. tile (g,k) at cols (g*8+k)*128.
    #      g: 0=Gz, 1=Gr, 2=Wh, 3=Uh; layout [in_feat_within_k(128), out_feat(128)]
    wg = nc.declare_dram_parameter("wg", [128, 4 * KT * 128], f16, isOutput=False)
    # u1:  step-0 z/r weights (Uz, Ur tiles), same tile layout, g: 0=Uz, 1=Ur
    u1 = nc.declare_dram_parameter("u1", [128, 2 * KT * 128], f16, isOutput=False)
    # ht0: initial transposed state fp16: [feat_in_block(128), slot(8)*batch(256)]
    ht0 = nc.declare_dram_parameter("ht0", [128, NCORES * B], f16, isOutput=False)
    # xt:  core's own fp32 state chunk [feat(128), batch(256)]
    xt = nc.declare_dram_parameter("xt", [128, B], f32, isOutput=False)
    if with_bias:
        bias = nc.declare_dram_parameter("bias", [128, 3], f32, isOutput=False)
    out = nc.declare_dram_parameter("out", [t_steps, 128, B], f32, isOutput=True)

    # ---- SBUF ----
    wg_sb = nc.alloc_sbuf_tensor("wg_sb", [128, 4 * KT * 128], f16)
    u1_sb = nc.alloc_sbuf_tensor("u1_sb", [128, 2 * KT * 128], f16)
    ht_sb = [nc.alloc_sbuf_tensor(f"ht{p}_sb", [128, NCORES * B], f16) for p in (0, 1)]
    h_sb = [nc.alloc_sbuf_tensor(f"h{p}_sb", [128, B], f32) for p in (0, 1)]
    zr_sb = nc.alloc_sbuf_tensor("zr_sb", [128, 2 * B], f32)   # z | r
    t1_sb = nc.alloc_sbuf_tensor("t1_sb", [128, B], f32)       # r * hl
    t2_sb = nc.alloc_sbuf_tensor("t2_sb", [128, B], f32)       # xh + r*hl
    hh_sb = nc.alloc_sbuf_tensor("hh_sb", [128, B], f32)       # tanh(...)
    f_sb = nc.alloc_sbuf_tensor("f_sb", [128, B], f32)         # z*h
    g1_sb = nc.alloc_sbuf_tensor("g1_sb", [128, B], f32)       # 1-z
    m_sb = nc.alloc_sbuf_tensor("m_sb", [128, B], f32)         # (1-z)*hh
    ones_sb = nc.alloc_sbuf_tensor("ones_sb", [128, B], f32)
    st_sb = [nc.alloc_sbuf_tensor(f"st{p}_sb", [128, B], f16) for p in (0, 1)]
    if with_bias:
        bias_sb = nc.alloc_sbuf_tensor("bias_sb", [128, 3], f32)

    # ---- PSUM (each [128,512]f32 = exactly one 2KB bank) ----
    psA = [nc.alloc_psum_tensor(f"psA{p}", [128, 2 * B], f32) for p in (0, 1)]  # z|r
    # xh and hl live in separate banks: DVE reads hl while the PE is still
    # accumulating xh, and same-bank PE-write + DVE-read is a hard fault.
    psB = [nc.alloc_psum_tensor(f"psB{p}", [128, 2 * B], f32) for p in (0, 1)]  # xh
    psC = [nc.alloc_psum_tensor(f"psC{p}", [128, 2 * B], f32) for p in (0, 1)]  # hl
    ps_junk = nc.alloc_psum_tensor("ps_junk", [128, 2 * B], f32)

    # ---- semaphores ----
    init_sem = nc.alloc_semaphore("init_sem")  # initial DMA loads (16/load)
    mm_sem = nc.alloc_semaphore("mm_sem")      # PE progress: +3 per step
    act_sem = nc.alloc_semaphore("act_sem")    # ACT progress: +2 per step
    dve_sem = nc.alloc_semaphore("dve_sem")    # DVE progress: +3 per step
    # one arrival semaphore per sender-pair (XOR distance k): +2 per step each.
    # A single accumulating sem would conflate steps: a fast peer's step-t+1
    # chunk could satisfy the step-t wait while a laggard's step-t chunk is
    # still in flight. Per-pair sems make the count per-sender exact.
    rsems = [nc.alloc_semaphore(f"rsem{k}") for k in range(NCORES)]
    bsem = nc.alloc_semaphore("bsem")          # local bcast-sent: +16 per step
    prep_sem = nc.alloc_semaphore("prep_sem")  # desc-gen done: +1 per step
    misc_sem = nc.alloc_semaphore("misc_sem")  # one-time init (ones memset)
    out_sem = nc.alloc_semaphore("out_sem")    # output DMA: +16 per step

    N_LOADS = 5 if with_bias else 4

    def wtile(g, k):
        return wg_sb[:, (g * KT + k) * 128:(g * KT + k + 1) * 128]

    def utile(g, k):
        return u1_sb[:, (g * KT + k) * 128:(g * KT + k + 1) * 128]

    with nc.Block() as block:

        @block.sync
        def _(sync):
            sync.dma_start(out=wg_sb[:, :], in_=wg[:, :]).then_inc(init_sem, 16)
            sync.dma_start(out=u1_sb[:, :], in_=u1[:, :]).then_inc(init_sem, 16)
            sync.dma_start(out=ht_sb[0][:, :], in_=ht0[:, :]).then_inc(init_sem, 16)
            sync.dma_start(out=h_sb[0][:, :], in_=xt[:, :]).then_inc(init_sem, 16)
            if with_bias:
                sync.dma_start(out=bias_sb[:, :], in_=bias[:, :]).then_inc(init_sem, 16)
            for t in range(t_steps):
                nxt = (t + 1) % 2
                # h(t+1) fp32 ready is the 3rd dve inc of step t (wait is
                # carried on the DMA instruction itself: every instruction
                # costs ~1.5us of dispatch on this runtime, so standalone
                # waits are folded into their consumers throughout)
                sync.dma_start(out=out[t], in_=h_sb[nxt][:, :]).then_inc(
                    out_sem, 16)._wait_ge(dve_sem, 3 * t + 3)

        @block.tensor
        def _(tensor):
            init_wait = [(init_sem, 16 * N_LOADS)]
            for t in range(t_steps):
                par, nxt = t % 2, (t + 1) % 2
                rhs = ht_sb[par]
                if t == 0:
                    # z/r from Uz/Ur; no xh (inp = 0); hl from Uh
                    for g, dst in ((0, psA[par][:, 0:B]), (1, psA[par][:, B:2 * B])):
                        for k in range(KT):
                            mm = tensor.matmul(
                                dst, utile(g, k), rhs[:, k * B:(k + 1) * B],
                                start=(k == 0), stop=(k == KT - 1))
                            if init_wait:
                                mm._wait_ge(*init_wait.pop())
                        if g == 1:
                            mm.then_inc(mm_sem, 1)
                    for k in range(KT):
                        mm = tensor.matmul(
                            psC[par][:, 0:B], wtile(3, k), rhs[:, k * B:(k + 1) * B],
                            start=(k == 0), stop=(k == KT - 1))
                    mm.then_inc(mm_sem, 2)
                else:
                    gdst = (
                        (0, psA[par][:, 0:B]),      # z
                        (1, psA[par][:, B:2 * B]),  # r
                        (3, psC[par][:, 0:B]),      # hl
                        (2, psB[par][:, 0:B]),      # xh
                    )
                    # Phase 1: k-tiles 0..3 slot-streamed — each slot's 4 gate
                    # MMs issue as soon as that slot's chunk lands, so the PE
                    # starts ~1us before the last chunks arrive (sends fire in
                    # slot order, so low slots land first). Groups interleave
                    # across the four psum banks, which is bank-safe.
                    for k in range(KT // 2):
                        # k=0 is the self slot: its data is this core's own
                        # st_sb (written by DVE at step t-1), so no loopback
                        # send exists for it and the gate is the local
                        # dve_sem, letting these 4 MMs start before any
                        # remote transfer lands.
                        krhs = (st_sb[nxt][:, :] if k == 0
                                else rhs[:, k * B:(k + 1) * B])
                        for gi, (g, dst) in enumerate(gdst):
                            # start=True clears has_written for the whole
                            # bank, so only the first gate touching each bank
                            # (z for psA, hl/xh for psC/psB) may set it; r's
                            # k0 write lands via overwrite-on-clear instead.
                            mm = tensor.matmul(
                                dst, wtile(g, k), krhs,
                                start=(k == 0 and g != 1), stop=False,
                                skip_group_check=True)
                            if gi == 0:
                                mm._wait_ge(*((dve_sem, 3 * t - 1) if k == 0
                                              else (rsems[k], 2 * t)))
                    # Phase 2: k-tiles 4..7 gate-major so z/r finish mid-PE
                    # and the sigmoid/t1 elementwise overlaps the hl/xh
                    # streams exactly as before.
                    for gi, (g, dst) in enumerate(gdst):
                        for k in range(KT // 2, KT):
                            mm = tensor.matmul(
                                dst, wtile(g, k), rhs[:, k * B:(k + 1) * B],
                                start=False, stop=(k == KT - 1),
                                skip_group_check=True)
                            if gi == 0:
                                mm._wait_ge(rsems[k], 2 * t)
                        if g != 0:
                            mm.then_inc(mm_sem, 1)  # after r, hl, xh

        @block.scalar
        def _(scalar):
            for t in range(t_steps):
                par = t % 2
                if with_bias:
                    scalar.activation(zr_sb[:, 0:B], psA[par][:, 0:B], Act.Sigmoid,
                                      bias=bias_sb[:, 0:1])._wait_ge(
                        mm_sem, 3 * t + 1)
                    sig = scalar.activation(zr_sb[:, B:2 * B], psA[par][:, B:2 * B],
                                            Act.Sigmoid, bias=bias_sb[:, 1:2])
                else:
                    sig = scalar.activation(zr_sb[:, 0:2 * B], psA[par][:, 0:2 * B],
                                            Act.Sigmoid)._wait_ge(mm_sem, 3 * t + 1)
                sig.then_inc(act_sem, 1)
                # tanh input: t=0 -> t1 (no xh term), else t2
                tin = t1_sb if t == 0 else t2_sb
                if with_bias:
                    th = scalar.activation(hh_sb[:, :], tin[:, :], Act.Tanh,
                                           bias=bias_sb[:, 2:3])
                else:
                    th = scalar.activation(hh_sb[:, :], tin[:, :], Act.Tanh)
                th._wait_ge(dve_sem, 3 * t + 1).then_inc(act_sem, 1)

        @block.vector
        def _(vector):
            for t in range(t_steps):
                par, nxt = t % 2, (t + 1) % 2
                # h' = z*h + (1-z)*hh. f and g1 depend only on z, so they run
                # while the PE is still streaming the hl/xh gates; after tanh
                # only two ops gate the fp16 send, and the fp32 state write is
                # off the critical path entirely.
                if t == 0:
                    vector.wait_ge(misc_sem, 1)  # ones_sb initialized
                vector.tensor_tensor(f_sb[:, :], zr_sb[:, 0:B], h_sb[par][:, :],
                                     Alu.mult)._wait_ge(act_sem, 2 * t + 1)
                vector.tensor_tensor(g1_sb[:, :], ones_sb[:, :], zr_sb[:, 0:B],
                                     Alu.subtract)
                # t1 = r * hl  (needs r from ACT, hl from PE)
                tt = vector.tensor_tensor(t1_sb[:, :], zr_sb[:, B:2 * B],
                                          psC[par][:, 0:B], Alu.mult)
                tt._wait_ge(mm_sem, 3 * t + 3 if t == 0 else 3 * t + 2)
                if t == 0:
                    tt.then_inc(dve_sem, 1)  # tanh input ready
                else:
                    vector.tensor_tensor(t2_sb[:, :], t1_sb[:, :], psB[par][:, 0:B],
                                         Alu.add)._wait_ge(
                        mm_sem, 3 * t + 3).then_inc(dve_sem, 1)
                vector.tensor_tensor(m_sb[:, :], g1_sb[:, :], hh_sb[:, :],
                                     Alu.mult)._wait_ge(act_sem, 2 * t + 2)
                if t >= 2:
                    # st_sb[par] was read by the 7 peer sends of step t-2.
                    # This wait value reaches ~14k — too wide for the fused
                    # on_wait immediate (the fused build passed T=8 but died
                    # at T=128), so it stays a standalone wait instruction.
                    vector.wait_ge(bsem, 16 * (NCORES - 1) * (t - 1))
                vector.tensor_tensor(st_sb[par][:, :], f_sb[:, :], m_sb[:, :],
                                     Alu.add).then_inc(dve_sem, 1)
                if t >= 2:
                    # h_sb[nxt] was DMA'd to out[t-2]; don't overwrite early
                    # (standalone for the same immediate-width reason)
                    vector.wait_ge(out_sem, 16 * (t - 1))
                vector.tensor_tensor(h_sb[nxt][:, :], f_sb[:, :], m_sb[:, :],
                                     Alu.add).then_inc(dve_sem, 1)

        @block.gpsimd
        def _(gpsimd):
            # Bacc's insert_library_loads switches the Q7 library for the
            # remote_dma instructions automatically.
            gpsimd.memset(ones_sb[:, :], 1.0).then_inc(misc_sem, 1)
            for t in range(t_steps):
                par, nxt = t % 2, (t + 1) % 2
                # (no rsem waits needed here: the dve_sem wait below already
                # transitively orders the sends after this core's PE consumed
                # the previous exchange)
                # 8 single-destination relative sends. Send k goes to the
                # physical-tpb XOR-k peer and lands at static slot k on the
                # receiver (register-offset APs hang the Q7 when several
                # preps are outstanding, so slots are compile-time). Slot j
                # on core r therefore holds the features of core
                # _slot_sender(r, j); the host permutes each core's weight
                # k-blocks and initial state to match. Each send has its own
                # pair semaphore rsems[k].
                # k=0 (self) is skipped: the PE reads st_sb directly for
                # its own k-tile, so only 7 peer sends are needed.
                for k in range(1, NCORES):
                    rdests = [None] * NCORES
                    rdests[k] = (0, k)
                    gpsimd.remote_dma_broadcast(
                        ht_sb[nxt][:, k * B:(k + 1) * B],
                        st_sb[par][:, :],
                        remote_sem=rsems[k],
                        local_sem=bsem,
                        rdests=rdests,
                    ).then_inc(prep_sem, 1)
                gpsimd.wait_ge(prep_sem, (NCORES - 1) * (t + 1))
                # fp16 chunk staged: wait carried on the trigger itself
                gpsimd.trigger_dma(NCORES - 1)._wait_ge(dve_sem, 3 * t + 2)

    nc.compile()
    return nc


# ---------------------------------------------------------------------------
# host side
# ---------------------------------------------------------------------------

# The trn2 driver maps logical NC i to physical NC _NC_BASE[i] (possibly
# XORed with a per-device mask, which cancels below). remote_dma's relative
# destinations XOR *physical* tpb ids, so the logical core whose chunk lands
# in slot k of logical core r is:
_NC_BASE = (0, 1, 2, 3, 6, 7, 4, 5)
_NC_BASE_INV = tuple(_NC_BASE.index(i) for i in range(8))


def _slot_sender(r, k):
    return _NC_BASE_INV[_NC_BASE[r] ^ k]


def _prep_inputs(x, W, U, b):
    """Build per-core input maps. Returns (in_maps, with_bias)."""
    x = np.asarray(x, np.float32)
    W = np.asarray(W, np.float32)
    U = np.asarray(U, np.float32)
    b = np.asarray(b, np.float32)
    with_bias = bool(np.any(b != 0.0))

    Wz, Wr, Wh = W[:, :D], W[:, D:2 * D], W[:, 2 * D:]
    Uz, Ur, Uh = U[:, :D], U[:, D:2 * D], U[:, 2 * D:]
    G = [Wz + Uz, Wr + Ur, Wh, Uh]          # steps >= 1 (inp == h)
    U1 = [Uz, Ur]                            # step 0 z/r (inp == 0)

    xt_all = x.T.reshape(NCORES, FB, B)  # [feat block, feat, batch]

    in_maps = []
    for c in range(NCORES):
        sl = slice(c * FB, (c + 1) * FB)
        # rhs slot j on core c holds the features of core _slot_sender(c, j),
        # so weight k-block j is that core's feature rows.
        perm = [_slot_sender(c, j) for j in range(NCORES)]
        # wg[p, (g*8+k)*128 + m] = G_g[perm[k]*128 + p, c*128 + m]
        wg = np.concatenate(
            [g[:, sl].reshape(KT, 128, FB)[perm[k]] for g in G for k in range(KT)],
            axis=1).astype(np.float16)
        u1 = np.concatenate(
            [g[:, sl].reshape(KT, 128, FB)[perm[k]] for g in U1 for k in range(KT)],
            axis=1).astype(np.float16)
        ht0 = np.ascontiguousarray(
            np.stack([xt_all[perm[j]] for j in range(NCORES)], axis=1)
            .reshape(FB, NCORES * B)).astype(np.float16)
        m = {
            "wg": np.ascontiguousarray(wg),
            "u1": np.ascontiguousarray(u1),
            "ht0": ht0,
            "xt": np.ascontiguousarray(x[:, sl].T),
        }
        if with_bias:
            m["bias"] = np.ascontiguousarray(
                np.stack([b[0 * D:1 * D][sl], b[1 * D:2 * D][sl],
                          b[2 * D:3 * D][sl]], axis=1))
        in_maps.append(m)
    return in_maps, with_bias


def run(x, W, U, b, trace=False, t_steps=T, **spmd_kwargs):
    import sys
    if "/opt/trn_rl_repo" not in sys.path:
        sys.path.insert(0, "/opt/trn_rl_repo")
    from concourse.bass_utils import run_bass_kernel_spmd

    in_maps, with_bias = _prep_inputs(x, W, U, b)
    nc = _build(t_steps, with_bias)
    res = run_bass_kernel_spmd(nc, in_maps, core_ids=list(range(NCORES)),
                               trace=trace, **spmd_kwargs)
    full = np.empty((B, t_steps, D), np.float32)
    for c in range(NCORES):
        co = np.asarray(res.results[c]["out"]).reshape(t_steps, FB, B)
        full[:, :, c * FB:(c + 1) * FB] = np.transpose(co, (2, 0, 1))
    return full, res


def kernel(x, W, U, b):
    return run(x, W, U, b)[0]



# revision 2
# speedup vs baseline: 1.0107x; 1.0107x over previous
"""Autoregressive GRU on 8 TRN2 NeuronCores — pair-sharded v2.

Problem: B=256, D=1024, T=128 decode steps.
  step:  z = sig(inp@Wz + h@Uz + bz); r = sig(inp@Wr + h@Ur + br)
         hh = tanh(inp@Wh + bh + r*(h@Uh));  h' = z*h + (1-z)*hh
  inp(0) = 0, h(0) = x, inp(t) == h(t) for t >= 1, so steps >= 1 use the
  fused Gz = Wz+Uz, Gr = Wr+Ur plus Wh, Uh ("xh"/"hl") separately.

Sharding: 4 batch quarters x 2 feature halves. Core c owns batch rows
[(c>>1)*64, +64) and feature half m = c&1 ([m*512, +512)). The pair
(c, c^1) shares a batch quarter; each step a core computes its 512
features for its 64 batch rows and exchanges the transposed fp16 state
chunk ([128, 256], 64 KB) with its XOR-1 partner — ONE remote send per
step per core. (The v1 layout needed 7 sends/step, and 7 outstanding
SWDGE preps cost ~42 us/step on the Q7; 1 prep + 1 trigger is ~2.7 us.)

Matmuls are batch-major: state tile hT[k] [128 feat, 64 batch] is
stationary, weight blocks stream as rhs [128, 512], accumulating
out[batch, feat] in PSUM. z/r share PSUM bank A (z on partitions 0:64,
r on 64:128 via the out-AP partition offset) so the sigmoid is a single
[128, 512] ACT op; hl and xh get their own banks (PE-write + DVE-read of
the same bank is a hard fault). Weight loads hide under the 512-cycle
streams (measured: matmul cost ~= stream cycles only, LDW is pulled
ahead by the PE's reorder window).

The new state [64 batch, 512 feat] fp16 is PE-transposed back to four
[128, 64] tiles for the next step's stationaries. 128 steps fully
unrolled, cross-engine order by explicit semaphores.
"""

import numpy as np

B = 256          # batch
D = 1024         # hidden
T = 128          # decode steps
NCORES = 8
BQ = B // 4      # batch rows per core = 64
FH = D // 2      # features per core = 512
KT = 8           # contraction k-tiles of 128


def _build(t_steps: int, with_bias: bool):
    import concourse.bass as bass
    import concourse.mybir as mybir
    from concourse import bacc

    f16 = mybir.dt.float16
    f32 = mybir.dt.float32
    Alu = mybir.AluOpType
    Act = mybir.ActivationFunctionType

    nc = bacc.Bacc()

    # ---- external I/O (per core) ----
    # wg: steps>=1 weights fp16. Block tt at cols tt*2048, layout
    #     [z(512) | r(512) | xh(512) | hl(512)]; rows = k-tile tt's global
    #     feature rows (tt 0-3 own half, 4-7 partner half).
    wg = nc.declare_dram_parameter("wg", [128, KT * 2048], f16, isOutput=False)
    # u1: step-0 weights [Uz(512) | Ur(512) | Uh(512)] per k-tile block.
    u1 = nc.declare_dram_parameter("u1", [128, KT * 1536], f16, isOutput=False)
    # ht0: initial transposed state fp16: 8 tiles [128 feat, 64 batch]
    ht0 = nc.declare_dram_parameter("ht0", [128, 512], f16, isOutput=False)
    # x32: initial own h slice fp32 [64 batch, 512 feat]
    x32 = nc.declare_dram_parameter("x32", [64, 512], f32, isOutput=False)
    ident = nc.declare_dram_parameter("ident", [64, 64], f16, isOutput=False)
    if with_bias:
        bias_a = nc.declare_dram_parameter("bias_a", [128, 512], f32, isOutput=False)
        bias_h = nc.declare_dram_parameter("bias_h", [64, 512], f32, isOutput=False)
    out = nc.declare_dram_parameter("out", [t_steps, 64, 512], f32, isOutput=True)

    # ---- SBUF ----
    wg_sb = nc.alloc_sbuf_tensor("wg_sb", [128, KT * 2048], f16)
    u1_sb = nc.alloc_sbuf_tensor("u1_sb", [128, KT * 1536], f16)
    # ht[p]: slot0 = own 4 tiles (cols 0:256), slot1 = partner (256:512)
    ht_sb = [nc.alloc_sbuf_tensor(f"ht{p}_sb", [128, 512], f16) for p in (0, 1)]
    h32_sb = [nc.alloc_sbuf_tensor(f"h32{p}_sb", [64, 512], f32) for p in (0, 1)]
    zr_sb = nc.alloc_sbuf_tensor("zr_sb", [128, 512], f32)  # z rows 0:64, r 64:128
    t1_sb = nc.alloc_sbuf_tensor("t1_sb", [64, 512], f32)
    t2_sb = nc.alloc_sbuf_tensor("t2_sb", [64, 512], f32)
    hh_sb = nc.alloc_sbuf_tensor("hh_sb", [64, 512], f32)
    d_sb = nc.alloc_sbuf_tensor("d_sb", [64, 512], f32)
    m_sb = nc.alloc_sbuf_tensor("m_sb", [64, 512], f32)
    h16_sb = nc.alloc_sbuf_tensor("h16_sb", [64, 512], f16)
    ident_sb = nc.alloc_sbuf_tensor("ident_sb", [64, 64], f16)
    if with_bias:
        zrb_sb = nc.alloc_sbuf_tensor("zrb_sb", [128, 512], f32)
        bias_a_sb = nc.alloc_sbuf_tensor("bias_a_sb", [128, 512], f32)
        bias_h_sb = nc.alloc_sbuf_tensor("bias_h_sb", [64, 512], f32)

    # ---- PSUM (bank-disjoint readers/writers) ----
    psA = [nc.alloc_psum_tensor(f"psA{p}", [128, 512], f32) for p in (0, 1)]
    psH = [nc.alloc_psum_tensor(f"psH{p}", [64, 512], f32) for p in (0, 1)]  # hl
    psX = [nc.alloc_psum_tensor(f"psX{p}", [64, 512], f32) for p in (0, 1)]  # xh
    psT = [nc.alloc_psum_tensor(f"psT{p}", [128, 256], f16) for p in (0, 1)]

    # ---- semaphores ----
    init_sem = nc.alloc_semaphore("init_sem")
    mm_sem = nc.alloc_semaphore("mm_sem")    # +4/step: A, hl, xh, transposes
    act_sem = nc.alloc_semaphore("act_sem")  # +3/step: sigmoid, tanh, slot0 copy
    dve_sem = nc.alloc_semaphore("dve_sem")  # +3/step: tanh-in, h16, h32
    rsem1 = nc.alloc_semaphore("rsem1")      # partner S1 arrivals: +2/step
    bsem = nc.alloc_semaphore("bsem")        # local send complete: +16/step
    prep_sem = nc.alloc_semaphore("prep_sem")
    out_sem = nc.alloc_semaphore("out_sem")
    if with_bias:
        zb_sem = nc.alloc_semaphore("zb_sem")

    N_LOADS = (7 if with_bias else 5)

    def htile(p, tt):
        return ht_sb[p][:, tt * 64:(tt + 1) * 64]

    with nc.Block() as block:

        @block.sync
        def _(sync):
            sync.dma_start(out=wg_sb[:, :], in_=wg[:, :]).then_inc(init_sem, 16)
            sync.dma_start(out=u1_sb[:, :], in_=u1[:, :]).then_inc(init_sem, 16)
            sync.dma_start(out=ht_sb[0][:, :], in_=ht0[:, :]).then_inc(init_sem, 16)
            sync.dma_start(out=h32_sb[0][:, :], in_=x32[:, :]).then_inc(init_sem, 16)
            sync.dma_start(out=ident_sb[:, :], in_=ident[:, :]).then_inc(init_sem, 16)
            if with_bias:
                sync.dma_start(out=bias_a_sb[:, :], in_=bias_a[:, :]).then_inc(init_sem, 16)
                sync.dma_start(out=bias_h_sb[:, :], in_=bias_h[:, :]).then_inc(init_sem, 16)
            for t in range(t_steps):
                nxt = (t + 1) % 2
                sync.dma_start(out=out[t], in_=h32_sb[nxt][:, :]).then_inc(
                    out_sem, 16)._wait_ge(dve_sem, 3 * t + 3)

        @block.tensor
        def _(tensor):
            for t in range(t_steps):
                par = t % 2
                rhs_w, blk = (u1_sb, 1536) if t == 0 else (wg_sb, 2048)
                # passes: (psum-ap-maker, col-offset-in-block, start, inc)
                if t == 0:
                    # u1 layout [Uz | Ur | Uh]; no xh pass, hl tail incs +2
                    passes = [
                        (psA[par][0:64, :], 0, True, 0),        # z
                        (psA[par][64:128, :], 512, True, 1),    # r -> A done
                        (psH[par][:, :], 1024, True, 2),        # hl (+2: no xh)
                    ]
                else:
                    passes = [
                        (psA[par][0:64, :], 0, True, 0),        # z
                        (psA[par][64:128, :], 512, True, 1),    # r -> A done
                        (psH[par][:, :], 1536, True, 1),        # hl
                        (psX[par][:, :], 1024, True, 1),        # xh
                    ]
                for pi, (ps_ap, co, start0, inc) in enumerate(passes):
                    for tt in range(KT):
                        mm = tensor.matmul(
                            ps_ap,
                            htile(par, tt),
                            rhs_w[:, tt * blk + co: tt * blk + co + 512],
                            start=(tt == 0 and start0),
                            stop=(tt == KT - 1),
                            skip_group_check=True)
                        if pi == 0 and tt == 0:
                            if t == 0:
                                mm._wait_ge(init_sem, 16 * N_LOADS)
                            else:
                                # own slot0 tiles: ACT copy of step t-1
                                mm._wait_ge(act_sem, 3 * t)
                        if pi == 0 and tt == 4 and t > 0:
                            # partner slot1 tiles: S1 of step t-1 landed
                            mm._wait_ge(rsem1, 2 * t)
                    if inc:
                        mm.then_inc(mm_sem, inc)
                # transposes: h16 [64, 512] -> four [128, 64] tiles
                for i in range(4):
                    tr = tensor.transpose(
                        psT[par][:, i * 64:(i + 1) * 64],
                        h16_sb[:, i * 128:(i + 1) * 128],
                        ident_sb[:, :])
                    if i == 0:
                        tr._wait_ge(dve_sem, 3 * t + 2)
                tr.then_inc(mm_sem, 1)

        @block.scalar
        def _(scalar):
            for t in range(t_steps):
                par, nxt = t % 2, (t + 1) % 2
                if with_bias:
                    sig = scalar.activation(zr_sb[:, :], zrb_sb[:, :],
                                            Act.Sigmoid)._wait_ge(zb_sem, t + 1)
                else:
                    sig = scalar.activation(zr_sb[:, :], psA[par][:, :],
                                            Act.Sigmoid)._wait_ge(mm_sem, 4 * t + 1)
                sig.then_inc(act_sem, 1)
                tin = t1_sb if t == 0 else t2_sb
                scalar.activation(hh_sb[:, :], tin[:, :], Act.Tanh)._wait_ge(
                    dve_sem, 3 * t + 1).then_inc(act_sem, 1)
                scalar.copy(ht_sb[nxt][:, 0:256], psT[par][:, :])._wait_ge(
                    mm_sem, 4 * t + 4).then_inc(act_sem, 1)

        @block.vector
        def _(vector):
            for t in range(t_steps):
                par, nxt = t % 2, (t + 1) % 2
                if with_bias:
                    # zrb = psA + bias (sigmoid input); done on DVE since the
                    # ACT bias operand is per-partition and batch-major bias
                    # varies along the free dim. b==0 compiles this out.
                    vector.tensor_tensor(zrb_sb[:, :], psA[par][:, :],
                                         bias_a_sb[:, :], Alu.add)._wait_ge(
                        mm_sem, 4 * t + 1).then_inc(zb_sem, 1)
                # t1 = r * hl (needs sigmoid AND the hl pass; one wait per
                # instruction, so the act wait is standalone)
                vector.wait_ge(act_sem, 3 * t + 1)
                tt1 = vector.tensor_tensor(
                    t1_sb[:, :], zr_sb[64:128, :], psH[par][:, :],
                    Alu.mult)._wait_ge(mm_sem, 4 * t + 2)
                if t == 0:
                    last = tt1
                    if with_bias:
                        last = vector.tensor_tensor(t1_sb[:, :], t1_sb[:, :],
                                                    bias_h_sb[:, :], Alu.add)
                else:
                    last = vector.tensor_tensor(
                        t2_sb[:, :], t1_sb[:, :], psX[par][:, :],
                        Alu.add)._wait_ge(mm_sem, 4 * t + 3)
                    if with_bias:
                        last = vector.tensor_tensor(t2_sb[:, :], t2_sb[:, :],
                                                    bias_h_sb[:, :], Alu.add)
                last.then_inc(dve_sem, 1)
                # d = h - hh ; m = z * d ; h' = hh + m
                vector.tensor_tensor(d_sb[:, :], h32_sb[par][:, :], hh_sb[:, :],
                                     Alu.subtract)._wait_ge(act_sem, 3 * t + 2)
                vector.tensor_tensor(m_sb[:, :], zr_sb[0:64, :], d_sb[:, :],
                                     Alu.mult)
                vector.tensor_tensor(h16_sb[:, :], hh_sb[:, :], m_sb[:, :],
                                     Alu.add).then_inc(dve_sem, 1)
                if t >= 2:
                    vector.wait_ge(out_sem, 16 * (t - 1))
                vector.tensor_tensor(h32_sb[nxt][:, :], hh_sb[:, :], m_sb[:, :],
                                     Alu.add).then_inc(dve_sem, 1)

        @block.gpsimd
        def _(gpsimd):
            for t in range(t_steps - 1):
                nxt = (t + 1) % 2
                rdests = [None] * NCORES
                rdests[1] = (0, 1)
                gpsimd.remote_dma_broadcast(
                    ht_sb[nxt][:, 256:512],
                    ht_sb[nxt][:, 0:256],
                    remote_sem=rsem1,
                    local_sem=bsem,
                    rdests=rdests,
                ).then_inc(prep_sem, 1)
                gpsimd.trigger_dma(1)._wait_ge(act_sem, 3 * t + 3)

    nc.compile()
    return nc


# ---------------------------------------------------------------------------
# host side
# ---------------------------------------------------------------------------

def _prep_inputs(x, W, U, b):
    x = np.asarray(x, np.float32)
    W = np.asarray(W, np.float32)
    U = np.asarray(U, np.float32)
    b = np.asarray(b, np.float32)
    with_bias = bool(np.any(b != 0.0))

    Wz, Wr, Wh = W[:, :D], W[:, D:2 * D], W[:, 2 * D:]
    Uz, Ur, Uh = U[:, :D], U[:, D:2 * D], U[:, 2 * D:]
    G = [Wz + Uz, Wr + Ur, Wh, Uh]   # z | r | xh | hl
    U1 = [Uz, Ur, Uh]

    in_maps = []
    for c in range(NCORES):
        m, p = c & 1, c >> 1
        bsl = slice(p * BQ, (p + 1) * BQ)
        fsl = slice(m * FH, (m + 1) * FH)
        wblocks, ublocks, hblocks = [], [], []
        for j in (0, 1):             # slot: 0 = own half, 1 = partner half
            half = m ^ j
            for i in range(4):       # k-tile within the half
                rows = slice(half * FH + i * 128, half * FH + (i + 1) * 128)
                wblocks.append(np.hstack([g[rows, fsl] for g in G]))
                ublocks.append(np.hstack([g[rows, fsl] for g in U1]))
                hblocks.append(x[bsl, rows].T)
        in_map = {
            "wg": np.ascontiguousarray(np.hstack(wblocks)).astype(np.float16),
            "u1": np.ascontiguousarray(np.hstack(ublocks)).astype(np.float16),
            "ht0": np.ascontiguousarray(np.hstack(hblocks)).astype(np.float16),
            "x32": np.ascontiguousarray(x[bsl, fsl]),
            "ident": np.eye(64, dtype=np.float16),
        }
        if with_bias:
            bz, br, bh = b[:D][fsl], b[D:2 * D][fsl], b[2 * D:][fsl]
            in_map["bias_a"] = np.ascontiguousarray(
                np.vstack([np.tile(bz, (64, 1)), np.tile(br, (64, 1))])
            ).astype(np.float32)
            in_map["bias_h"] = np.ascontiguousarray(
                np.tile(bh, (64, 1))).astype(np.float32)
        in_maps.append(in_map)
    return in_maps, with_bias


def gather(results, t_steps=T):
    """results: per-core dicts with 'out' [t_steps, 64, 512] -> [B, T, D]."""
    full = np.empty((B, t_steps, D), np.float32)
    for c in range(NCORES):
        m, p = c & 1, c >> 1
        co = np.asarray(results[c]["out"]).reshape(t_steps, BQ, FH)
        full[p * BQ:(p + 1) * BQ, :, m * FH:(m + 1) * FH] = \
            np.transpose(co, (1, 0, 2))
    return full


def run(x, W, U, b, trace=False, t_steps=T, **spmd_kwargs):
    import sys
    if "/opt/trn_rl_repo" not in sys.path:
        sys.path.insert(0, "/opt/trn_rl_repo")
    from concourse.bass_utils import run_bass_kernel_spmd

    in_maps, with_bias = _prep_inputs(x, W, U, b)
    nc = _build(t_steps, with_bias)
    res = run_bass_kernel_spmd(nc, in_maps, core_ids=list(range(NCORES)),
                               trace=trace, **spmd_kwargs)
    full = np.empty((B, t_steps, D), np.float32)
    for c in range(NCORES):
        m, p = c & 1, c >> 1
        co = np.asarray(res.results[c]["out"])  # [T, 64, 512]
        full[p * BQ:(p + 1) * BQ, :, m * FH:(m + 1) * FH] = \
            np.transpose(co, (1, 0, 2))
    return full, res


def kernel(x, W, U, b):
    return run(x, W, U, b)[0]
